# revision 16
# baseline (speedup 1.0000x reference)
"""Trainium2 Bass kernel for CRF negative log-likelihood (nn_CRF_33913061769500).

Strategy (pure data parallel over batch, 8 cores x 16 sequences):
  Forward scores are computed in the exp domain: alpha_t = c_t + log p_t with
  p_{t+1} = (Eexp^T p_t) * exp(feats_t - GAMMA), Eexp = exp(transitions).
  The matmul contracts over the "from" tag on the partition axis (bf16,
  single-pass); emissions multiply elementwise on VectorE. Rare
  renormalizations (t=63/127/191) keep p in fp32 range; their log-scales ship
  to the host. Terminal scores for all t come from a per-band matmul with
  exp(trans[:,STOP]) against the stored p history; a host-built one-hot over
  t = len-1 selects each sequence's value, so the mask never needs applying
  inside the scan. The gold emission score uses an iota/one-hot gather
  on-device; the tiny tags/transitions-only gold terms fold in on the host.
"""
import sys

import numpy as np

try:
    import concourse.bass as bass
except ImportError:  # pragma: no cover
    sys.path.insert(0, "/opt/trn_rl_repo")
    import concourse.bass as bass

import concourse.bacc as bacc
import concourse.mybir as mybir
import concourse.tile as tile
from concourse.bass_utils import run_bass_kernel_spmd

F32 = mybir.dt.float32
BF16 = mybir.dt.bfloat16
I32 = mybir.dt.int32
AF = mybir.ActivationFunctionType
ALU = mybir.AluOpType

B, S, T = 128, 256, 52
START, STOP, PAD = 50, 51, 0
NCORES = 8
BL = B // NCORES          # 16 sequences per core
GAMMA = 4.0
NORM_TS = (63, 127, 191)  # renormalize after computing p_t at these t
NBANDS = 8                # W computed per band of 32 timesteps
BAND = S // NBANDS        # 32


def build_program(stage=99):
    nc = bacc.Bacc("TRN2", target_bir_lowering=False, debug=False)

    feats_d = nc.dram_tensor("feats128", [128, 1664], F32, kind="ExternalInput")
    tags_d = nc.dram_tensor("tags128", [128, 32], I32, kind="ExternalInput")
    packf_d = nc.dram_tensor("packf", [128, 132], F32, kind="ExternalInput")
    packb_d = nc.dram_tensor("packb", [128, 108], BF16, kind="ExternalInput")
    sel_d = nc.dram_tensor("sel", [1, S * BL], F32, kind="ExternalInput")

    o_selw = nc.dram_tensor("out_selw", [BL], F32, kind="ExternalOutput")
    o_logs = nc.dram_tensor("out_logs", [3 * BL], F32, kind="ExternalOutput")
    o_gold = nc.dram_tensor("out_gold", [128], F32, kind="ExternalOutput")
    o_dbg = nc.dram_tensor("out_dbg", [128, 16], F32, kind="ExternalOutput")

    with tile.TileContext(nc) as tc:
        with (
            tc.tile_pool(name="sb", bufs=1) as sb,
            tc.tile_pool(name="ptp", bufs=2, space="PSUM") as ptp,
            tc.tile_pool(name="pq", bufs=1, space="PSUM") as pq,
            tc.tile_pool(name="pn", bufs=1, space="PSUM") as pn,
            tc.tile_pool(name="pw", bufs=2, space="PSUM") as pw,
        ):
            # ---- constants (two consolidated packs + sel + tags) ----
            packf = sb.tile([128, 132], F32)
            nc.gpsimd.dma_start(out=packf[:, :], in_=packf_d[:, :])
            packb = sb.tile([128, 108], BF16)
            nc.gpsimd.dma_start(out=packb[:, :], in_=packb_d[:, :])
            tags_sb = sb.tile([128, 32], I32)
            nc.gpsimd.dma_start(out=tags_sb[:, :], in_=tags_d[:, :])
            sel_sb = sb.tile([1, S * BL], F32)
            nc.gpsimd.dma_start(out=sel_sb[:, :], in_=sel_d[:, :])

            ident = packf[:, 0:128]
            estart = packf[0:T, 128:129]
            gbias = packf[0:T, 129:130]
            eexp_b = packb[0:T, 0:T]
            estop_b = packb[0:T, T:T + 1]
            onescol_b = packb[0:T, 53:54]
            onesrow_b = packb[0:1, 54:106]

            # ---- feats: 8 column-chunk DMAs so transposes can start early ----
            feats_sb = sb.tile([128, 1664], F32)
            for ci in range(8):
                nc.sync.dma_start(
                    out=feats_sb[:, ci * 208:(ci + 1) * 208],
                    in_=feats_d[:, ci * 208:(ci + 1) * 208],
                )

            if stage <= 1:
                nc.sync.dma_start(out=o_dbg[:, :], in_=feats_sb[:, 0:16])

            # ---- transpose feats -> expf[j, v*128 + b*8 + tc] = exp(f - GAMMA)
            expf = sb.tile([T, 32 * 128], BF16)
            for g in range(8 if stage >= 2 else 0):
                tp = ptp.tile([T, 512], F32, tag="tp")
                for k in range(4):
                    v = 4 * g + k
                    nc.tensor.transpose(
                        tp[0:T, k * 128:(k + 1) * 128],
                        feats_sb[:, v * T:(v + 1) * T],
                        ident,
                    )
                nc.scalar.activation(
                    expf[0:T, g * 512:(g + 1) * 512], tp[0:T, :], AF.Exp,
                    bias=gbias,
                )

            # view: (j, v, b, tc) with t = tc*32 + v
            expf_r = expf[0:T, :].rearrange("p (v b c) -> p v b c", v=32, b=BL, c=8)

            if stage == 2:
                nc.sync.dma_start(out=o_dbg[0:T, :], in_=expf[0:T, 0:16])

            # ---- gold emission gather (independent of scan) ----
            do_gold = (stage >= 3 and stage != 4) or stage in (31, 32)
            jiota = sb.tile([128, 1664], I32)
            onehot = sb.tile([128, 1664], F32)
            goldscr = sb.tile([128, 1664], F32)
            goldp = sb.tile([128, 1], F32)
            if do_gold:
                nc.gpsimd.iota(
                    jiota[:, :].rearrange("p (t j) -> p t j", t=32, j=T),
                    pattern=[[0, 32], [1, T]],
                    base=0,
                    channel_multiplier=0,
                )
                nc.vector.tensor_tensor(
                    out=onehot[:, :].rearrange("p (t j) -> p t j", t=32, j=T),
                    in0=jiota[:, :].rearrange("p (t j) -> p t j", t=32, j=T),
                    in1=tags_sb[:, :].broadcast_to((128, 32, T)),
                    op=ALU.is_equal,
                )
                nc.vector.tensor_tensor(
                    out=goldscr[:, :], in0=onehot[:, :], in1=feats_sb[:, :],
                    op=ALU.mult,
                )
                nc.vector.tensor_reduce(
                    goldp[:, :], goldscr[:, :],
                    axis=mybir.AxisListType.X, op=ALU.add,
                )
                nc.sync.dma_start(out=o_gold[:], in_=goldp[:, :])

            # ---- scan ----
            pall = sb.tile([T, S * BL], BF16)
            logs_sb = sb.tile([1, 3 * BL], F32)
            recip_sb = sb.tile([1, BL], F32)
            recipb_sb = sb.tile([1, BL], BF16)
            wpart = sb.tile([1, NBANDS * BL], F32)
            selw_sb = sb.tile([1, BL], F32)

            do_scan = stage >= 4
            do_norm = stage >= 5
            do_w = stage >= 6

            # p_0 = exp(feats_0 - GAMMA) * exp(trans[START, :])
            if do_scan:
                nc.vector.tensor_scalar_mul(
                    pall[0:T, 0:BL], expf_r[0:T, 0, 0:BL, 0], estart
                )

            for t in range(1, S if do_scan else 0):
                v, c = t % 32, t // 32
                for h in range(2):
                    b0 = 8 * h
                    q = pq.tile([T, 8], F32, tag=f"q{h}")
                    nc.tensor.matmul(
                        q[:, :],
                        eexp_b,
                        pall[0:T, (t - 1) * BL + b0:(t - 1) * BL + b0 + 8],
                        start=True, stop=True,
                    )
                    nc.vector.tensor_tensor(
                        out=pall[0:T, t * BL + b0:t * BL + b0 + 8],
                        in0=q[:, :],
                        in1=expf_r[0:T, v, b0:b0 + 8, c],
                        op=ALU.mult,
                    )

                if t in NORM_TS and do_norm:
                    k = NORM_TS.index(t)
                    sn = pn.tile([1, BL], F32, tag="sn")
                    nc.tensor.matmul(
                        sn[0:1, :], onescol_b,
                        pall[0:T, t * BL:(t + 1) * BL],
                        start=True, stop=True,
                    )
                    nc.vector.reciprocal(recip_sb[0:1, :], sn[0:1, :])
                    nc.scalar.copy(recipb_sb[0:1, :], recip_sb[0:1, :])
                    bc = pn.tile([T, BL], F32, tag="bc")
                    nc.tensor.matmul(
                        bc[:, :], onesrow_b, recipb_sb[0:1, :],
                        start=True, stop=True,
                    )
                    nc.vector.tensor_tensor(
                        out=pall[0:T, t * BL:(t + 1) * BL],
                        in0=bc[:, :],
                        in1=pall[0:T, t * BL:(t + 1) * BL],
                        op=ALU.mult,
                    )
                    # ship log(applied recip scale); host subtracts from c
                    nc.scalar.activation(
                        logs_sb[0:1, k * BL:(k + 1) * BL], recipb_sb[0:1, :],
                        AF.Ln,
                    )

                if t % BAND == BAND - 1 and do_w:
                    n = t // BAND
                    wq = pw.tile([1, BAND * BL], F32, tag="wq")
                    nc.tensor.matmul(
                        wq[0:1, :], estop_b,
                        pall[0:T, n * BAND * BL:(n + 1) * BAND * BL],
                        start=True, stop=True,
                    )
                    wsel = sb.tile([1, BAND * BL], F32, tag="wsel", bufs=2)
                    nc.vector.tensor_tensor(
                        out=wsel[0:1, :],
                        in0=wq[0:1, :],
                        in1=sel_sb[0:1, n * BAND * BL:(n + 1) * BAND * BL],
                        op=ALU.mult,
                    )
                    nc.vector.tensor_reduce(
                        wpart[0:1, n * BL:(n + 1) * BL],
                        wsel[0:1, :].rearrange("p (t b) -> p b t", t=BAND, b=BL),
                        axis=mybir.AxisListType.X,
                        op=ALU.add,
                    )

            if do_scan and not do_w:
                nc.scalar.copy(goldscr[0:T, 0:16], pall[0:T, 100 * BL:101 * BL])
                nc.sync.dma_start(out=o_dbg[0:T, :], in_=goldscr[0:T, 0:16])
            if do_w:
                nc.vector.tensor_reduce(
                    selw_sb[0:1, :],
                    wpart[0:1, :].rearrange("p (n b) -> p b n", n=NBANDS, b=BL),
                    axis=mybir.AxisListType.X,
                    op=ALU.add,
                )
                nc.sync.dma_start(out=o_selw[:], in_=selw_sb[0:1, :])
                nc.sync.dma_start(out=o_logs[:], in_=logs_sb[0:1, :])

    nc.compile()
    return nc


def host_prep(feats, transitions, mask, tags):
    """Per-core input maps + host-side gold constants."""
    import ml_dtypes
    bf = ml_dtypes.bfloat16

    feats = np.ascontiguousarray(np.asarray(feats, np.float32))
    trans = np.asarray(transitions, np.float32)
    mask = np.asarray(mask).astype(bool)
    tags = np.asarray(tags, np.int32)

    lengths = mask.sum(1).astype(np.int64)                      # (B,)
    eexp = np.exp(trans).astype(np.float32)
    tags_m = np.where(mask, tags, -1).astype(np.int32)

    packf = np.zeros((128, 132), np.float32)
    packf[:, 0:128] = np.eye(128, dtype=np.float32)
    packf[0:T, 128] = np.exp(trans[START]).astype(np.float32)
    packf[0:T, 129] = -GAMMA

    packb = np.zeros((128, 108), np.float32)
    packb[0:T, 0:T] = eexp
    packb[0:T, T] = np.exp(trans[:, STOP]).astype(np.float32)
    packb[0:T, 53] = 1.0
    packb[:, 54:106] = 1.0
    packb = packb.astype(bf)

    in_maps = []
    for core in range(NCORES):
        bs = slice(BL * core, BL * (core + 1))
        f128 = feats[bs].reshape(128, 1664)        # (b,tc,tl,j)->(b*8+tc, tl*52+j)
        t128 = tags_m[bs].reshape(128, 32)
        L = lengths[bs]
        sel = np.zeros((1, S * BL), np.float32)
        for b in range(BL):
            sel[0, (L[b] - 1) * BL + b] = 1.0
        in_maps.append({
            "feats128": f128,
            "tags128": t128,
            "packf": packf,
            "packb": packb,
            "sel": sel,
        })

    # gold terms that only involve tags/transitions/mask (tiny host math)
    prev = np.concatenate([np.full((B, 1), START, tags.dtype), tags[:, :-1]], 1)
    trans_gold = trans[prev, tags]
    end_ids = tags[np.arange(B), lengths - 1]
    gold_host = np.where(mask, trans_gold, 0.0).sum(dtype=np.float64) \
        + trans[end_ids, STOP].sum(dtype=np.float64)
    return in_maps, lengths, gold_host


def postprocess(results, lengths, gold_host):
    total = np.float64(0.0)
    for core in range(NCORES):
        r = results[core]
        L = lengths[BL * core:BL * (core + 1)].astype(np.float64)
        selw = np.asarray(r["out_selw"], np.float64)
        logs = np.asarray(r["out_logs"], np.float64).reshape(3, BL)
        gold = np.asarray(r["out_gold"], np.float64)
        cfin = GAMMA * L
        for k, tk in enumerate(NORM_TS):
            # logs holds log(recip) of the applied rescale; c -= log(recip)
            cfin -= (L - 1 >= tk) * logs[k]
        term = cfin + np.log(selw)
        total += term.sum() - gold.sum()
    total -= gold_host
    return np.float32(total)


_NC_CACHE = {}


def _get_nc():
    if "nc" not in _NC_CACHE:
        _NC_CACHE["nc"] = build_program()
    return _NC_CACHE["nc"]


def kernel(feats, transitions, mask, tags, _trace=False):
    in_maps, lengths, gold_host = host_prep(feats, transitions, mask, tags)
    nc = _get_nc()
    res = run_bass_kernel_spmd(nc, in_maps, list(range(NCORES)), trace=_trace)
    out = postprocess(res.results, lengths, gold_host)
    if _trace:
        return out, res
    return out


# revision 18
# speedup vs baseline: 1.0889x; 1.0889x over previous
"""Trainium2 Bass kernel for CRF negative log-likelihood (nn_CRF_33913061769500).

Strategy (pure data parallel over batch, 8 cores x 16 sequences):
  Forward scores are computed in the exp domain: alpha_t = c_t + log p_t with
  p_{t+1} = (Eexp^T p_t) * exp(feats_t - GAMMA), Eexp = exp(transitions).
  The matmul contracts over the "from" tag on the partition axis (bf16,
  single-pass); emissions multiply elementwise on VectorE. Rare
  renormalizations (t=63/127/191) keep p in fp32 range; their log-scales ship
  to the host. Terminal scores for all t come from a per-band matmul with
  exp(trans[:,STOP]) against the stored p history; a host-built one-hot over
  t = len-1 selects each sequence's value, so the mask never needs applying
  inside the scan. The gold emission score uses an iota/one-hot gather
  on-device; the tiny tags/transitions-only gold terms fold in on the host.
"""
import sys

import numpy as np

try:
    import concourse.bass as bass
except ImportError:  # pragma: no cover
    sys.path.insert(0, "/opt/trn_rl_repo")
    import concourse.bass as bass

import concourse.bacc as bacc
import concourse.mybir as mybir
import concourse.tile as tile
from concourse.bass_utils import run_bass_kernel_spmd

F32 = mybir.dt.float32
BF16 = mybir.dt.bfloat16
I32 = mybir.dt.int32
AF = mybir.ActivationFunctionType
ALU = mybir.AluOpType

B, S, T = 128, 256, 52
START, STOP, PAD = 50, 51, 0
NCORES = 8
BL = B // NCORES          # 16 sequences per core
GAMMA = 4.0
NORM_TS = (63, 127, 191)  # renormalize after computing p_t at these t
NBANDS = 8                # W computed per band of 32 timesteps
BAND = S // NBANDS        # 32


def build_program(stage=99):
    nc = bacc.Bacc("TRN2", target_bir_lowering=False, debug=False)

    feats_d = nc.dram_tensor("feats128", [128, 1664], F32, kind="ExternalInput")
    tags_d = nc.dram_tensor("tags128", [128, 32], I32, kind="ExternalInput")
    packf_d = nc.dram_tensor("packf", [128, 132], F32, kind="ExternalInput")
    packb_d = nc.dram_tensor("packb", [128, 108], BF16, kind="ExternalInput")
    o_w = nc.dram_tensor("out_w", [S * BL], F32, kind="ExternalOutput")
    o_logs = nc.dram_tensor("out_logs", [3 * BL], F32, kind="ExternalOutput")
    o_gold = nc.dram_tensor("out_gold", [128], F32, kind="ExternalOutput")
    o_dbg = nc.dram_tensor("out_dbg", [128, 16], F32, kind="ExternalOutput")

    with tile.TileContext(nc) as tc:
        with (
            tc.tile_pool(name="sb", bufs=1) as sb,
            tc.tile_pool(name="ptp", bufs=2, space="PSUM") as ptp,
            tc.tile_pool(name="pq", bufs=1, space="PSUM") as pq,
            tc.tile_pool(name="pn", bufs=1, space="PSUM") as pn,
            tc.tile_pool(name="pw", bufs=2, space="PSUM") as pw,
        ):
            # ---- constants (two consolidated packs + sel + tags) ----
            packf = sb.tile([128, 132], F32)
            nc.gpsimd.dma_start(out=packf[:, :], in_=packf_d[:, :])
            packb = sb.tile([128, 108], BF16)
            nc.gpsimd.dma_start(out=packb[:, :], in_=packb_d[:, :])
            tags_sb = sb.tile([128, 32], I32)
            nc.gpsimd.dma_start(out=tags_sb[:, :], in_=tags_d[:, :])

            ident = packf[:, 0:128]
            estart = packf[0:T, 128:129]
            gbias = packf[0:T, 129:130]
            eexp_b = packb[0:T, 0:T]
            estop_b = packb[0:T, T:T + 1]
            onescol_b = packb[0:T, 53:54]
            onesrow_b = packb[0:1, 54:106]

            # ---- feats: 8 column-chunk DMAs so transposes can start early ----
            feats_sb = sb.tile([128, 1664], F32)
            for ci in range(8):
                nc.sync.dma_start(
                    out=feats_sb[:, ci * 208:(ci + 1) * 208],
                    in_=feats_d[:, ci * 208:(ci + 1) * 208],
                )

            if stage <= 1:
                nc.sync.dma_start(out=o_dbg[:, :], in_=feats_sb[:, 0:16])

            # ---- transpose feats -> expf[j, v*128 + b*8 + tc] = exp(f - GAMMA)
            expf = sb.tile([T, 32 * 128], BF16)
            for g in range(8 if stage >= 2 else 0):
                tp = ptp.tile([T, 512], F32, tag="tp")
                for k in range(4):
                    v = 4 * g + k
                    nc.tensor.transpose(
                        tp[0:T, k * 128:(k + 1) * 128],
                        feats_sb[:, v * T:(v + 1) * T],
                        ident,
                    )
                nc.scalar.activation(
                    expf[0:T, g * 512:(g + 1) * 512], tp[0:T, :], AF.Exp,
                    bias=gbias,
                )

            # view: (j, v, b, tc) with t = tc*32 + v
            expf_r = expf[0:T, :].rearrange("p (v b c) -> p v b c", v=32, b=BL, c=8)

            if stage == 2:
                nc.sync.dma_start(out=o_dbg[0:T, :], in_=expf[0:T, 0:16])

            # ---- gold emission gather (independent of scan) ----
            do_gold = (stage >= 3 and stage != 4) or stage in (31, 32)
            jiota = sb.tile([128, 1664], I32)
            onehot = sb.tile([128, 1664], F32)
            goldscr = sb.tile([128, 1664], F32)
            goldp = sb.tile([128, 1], F32)
            if do_gold:
                nc.gpsimd.iota(
                    jiota[:, :].rearrange("p (t j) -> p t j", t=32, j=T),
                    pattern=[[0, 32], [1, T]],
                    base=0,
                    channel_multiplier=0,
                )
                nc.vector.tensor_tensor(
                    out=onehot[:, :].rearrange("p (t j) -> p t j", t=32, j=T),
                    in0=jiota[:, :].rearrange("p (t j) -> p t j", t=32, j=T),
                    in1=tags_sb[:, :].broadcast_to((128, 32, T)),
                    op=ALU.is_equal,
                )
                nc.vector.tensor_tensor(
                    out=goldscr[:, :], in0=onehot[:, :], in1=feats_sb[:, :],
                    op=ALU.mult,
                )
                nc.vector.tensor_reduce(
                    goldp[:, :], goldscr[:, :],
                    axis=mybir.AxisListType.X, op=ALU.add,
                )
                nc.sync.dma_start(out=o_gold[:], in_=goldp[:, :])

            # ---- scan ----
            pall = sb.tile([T, S * BL], BF16)
            logs_sb = sb.tile([1, 3 * BL], F32)
            recip_sb = sb.tile([1, BL], F32)
            recipb_sb = sb.tile([1, BL], BF16)
            wbuf = sb.tile([1, S * BL], F32)

            do_scan = stage >= 4
            do_norm = stage >= 5
            do_w = stage >= 6

            # p_0 = exp(feats_0 - GAMMA) * exp(trans[START, :])
            if do_scan:
                nc.vector.tensor_scalar_mul(
                    pall[0:T, 0:BL], expf_r[0:T, 0, 0:BL, 0], estart
                )

            for t in range(1, S if do_scan else 0):
                v, c = t % 32, t // 32
                for h in range(2):
                    b0 = 8 * h
                    q = pq.tile([T, 8], F32, tag=f"q{h}")
                    nc.tensor.matmul(
                        q[:, :],
                        eexp_b,
                        pall[0:T, (t - 1) * BL + b0:(t - 1) * BL + b0 + 8],
                        start=True, stop=True,
                    )
                    nc.vector.tensor_tensor(
                        out=pall[0:T, t * BL + b0:t * BL + b0 + 8],
                        in0=q[:, :],
                        in1=expf_r[0:T, v, b0:b0 + 8, c],
                        op=ALU.mult,
                    )

                if t in NORM_TS and do_norm:
                    k = NORM_TS.index(t)
                    sn = pn.tile([1, BL], F32, tag="sn")
                    nc.tensor.matmul(
                        sn[0:1, :], onescol_b,
                        pall[0:T, t * BL:(t + 1) * BL],
                        start=True, stop=True,
                    )
                    nc.vector.reciprocal(recip_sb[0:1, :], sn[0:1, :])
                    nc.scalar.copy(recipb_sb[0:1, :], recip_sb[0:1, :])
                    bc = pn.tile([T, BL], F32, tag="bc")
                    nc.tensor.matmul(
                        bc[:, :], onesrow_b, recipb_sb[0:1, :],
                        start=True, stop=True,
                    )
                    nc.vector.tensor_tensor(
                        out=pall[0:T, t * BL:(t + 1) * BL],
                        in0=bc[:, :],
                        in1=pall[0:T, t * BL:(t + 1) * BL],
                        op=ALU.mult,
                    )
                    # ship log(applied recip scale); host subtracts from c
                    nc.scalar.activation(
                        logs_sb[0:1, k * BL:(k + 1) * BL], recipb_sb[0:1, :],
                        AF.Ln,
                    )

                if t % BAND == BAND - 1 and do_w:
                    n = t // BAND
                    wq = pw.tile([1, BAND * BL], F32, tag="wq")
                    nc.tensor.matmul(
                        wq[0:1, :], estop_b,
                        pall[0:T, n * BAND * BL:(n + 1) * BAND * BL],
                        start=True, stop=True,
                    )
                    nc.scalar.copy(
                        wbuf[0:1, n * BAND * BL:(n + 1) * BAND * BL],
                        wq[0:1, :],
                    )

            if do_scan and not do_w:
                nc.scalar.copy(goldscr[0:T, 0:16], pall[0:T, 100 * BL:101 * BL])
                nc.sync.dma_start(out=o_dbg[0:T, :], in_=goldscr[0:T, 0:16])
            if do_w:
                nc.sync.dma_start(out=o_w[:], in_=wbuf[0:1, :])
                nc.sync.dma_start(out=o_logs[:], in_=logs_sb[0:1, :])

    nc.compile()
    return nc


def host_prep(feats, transitions, mask, tags):
    """Per-core input maps + host-side gold constants."""
    import ml_dtypes
    bf = ml_dtypes.bfloat16

    feats = np.ascontiguousarray(np.asarray(feats, np.float32))
    trans = np.asarray(transitions, np.float32)
    mask = np.asarray(mask).astype(bool)
    tags = np.asarray(tags, np.int32)

    lengths = mask.sum(1).astype(np.int64)                      # (B,)
    eexp = np.exp(trans).astype(np.float32)
    tags_m = np.where(mask, tags, -1).astype(np.int32)

    packf = np.zeros((128, 132), np.float32)
    packf[:, 0:128] = np.eye(128, dtype=np.float32)
    packf[0:T, 128] = np.exp(trans[START]).astype(np.float32)
    packf[0:T, 129] = -GAMMA

    packb = np.zeros((128, 108), np.float32)
    packb[0:T, 0:T] = eexp
    packb[0:T, T] = np.exp(trans[:, STOP]).astype(np.float32)
    packb[0:T, 53] = 1.0
    packb[:, 54:106] = 1.0
    packb = packb.astype(bf)

    in_maps = []
    for core in range(NCORES):
        bs = slice(BL * core, BL * (core + 1))
        f128 = feats[bs].reshape(128, 1664)        # (b,tc,tl,j)->(b*8+tc, tl*52+j)
        t128 = tags_m[bs].reshape(128, 32)
        in_maps.append({
            "feats128": f128,
            "tags128": t128,
            "packf": packf,
            "packb": packb,
        })

    # gold terms that only involve tags/transitions/mask (tiny host math)
    prev = np.concatenate([np.full((B, 1), START, tags.dtype), tags[:, :-1]], 1)
    trans_gold = trans[prev, tags]
    end_ids = tags[np.arange(B), lengths - 1]
    gold_host = np.where(mask, trans_gold, 0.0).sum(dtype=np.float64) \
        + trans[end_ids, STOP].sum(dtype=np.float64)
    return in_maps, lengths, gold_host


def postprocess(results, lengths, gold_host):
    total = np.float64(0.0)
    for core in range(NCORES):
        r = results[core]
        L = lengths[BL * core:BL * (core + 1)].astype(np.float64)
        w = np.asarray(r["out_w"], np.float64).reshape(S, BL)
        selw = w[L.astype(np.int64) - 1, np.arange(BL)]
        logs = np.asarray(r["out_logs"], np.float64).reshape(3, BL)
        gold = np.asarray(r["out_gold"], np.float64)
        cfin = GAMMA * L
        for k, tk in enumerate(NORM_TS):
            # logs holds log(recip) of the applied rescale; c -= log(recip)
            cfin -= (L - 1 >= tk) * logs[k]
        term = cfin + np.log(selw)
        total += term.sum() - gold.sum()
    total -= gold_host
    return np.float32(total)


_NC_CACHE = {}


def _get_nc():
    if "nc" not in _NC_CACHE:
        _NC_CACHE["nc"] = build_program()
    return _NC_CACHE["nc"]


def kernel(feats, transitions, mask, tags, _trace=False):
    in_maps, lengths, gold_host = host_prep(feats, transitions, mask, tags)
    nc = _get_nc()
    res = run_bass_kernel_spmd(nc, in_maps, list(range(NCORES)), trace=_trace)
    out = postprocess(res.results, lengths, gold_host)
    if _trace:
        return out, res
    return out


# revision 19
# speedup vs baseline: 1.1205x; 1.0291x over previous
"""Trainium2 Bass kernel for CRF negative log-likelihood (nn_CRF_33913061769500).

Strategy (pure data parallel over batch, 8 cores x 16 sequences):
  Forward scores are computed in the exp domain: alpha_t = c_t + log p_t with
  p_{t+1} = (Eexp^T p_t) * exp(feats_t - GAMMA), Eexp = exp(transitions).
  The matmul contracts over the "from" tag on the partition axis (bf16,
  single-pass); emissions multiply elementwise on VectorE. Rare
  renormalizations (t=63/127/191) keep p in fp32 range; their log-scales ship
  to the host. Terminal scores for all t come from a per-band matmul with
  exp(trans[:,STOP]) against the stored p history; a host-built one-hot over
  t = len-1 selects each sequence's value, so the mask never needs applying
  inside the scan. The gold emission score uses an iota/one-hot gather
  on-device; the tiny tags/transitions-only gold terms fold in on the host.
"""
import sys

import numpy as np

try:
    import concourse.bass as bass
except ImportError:  # pragma: no cover
    sys.path.insert(0, "/opt/trn_rl_repo")
    import concourse.bass as bass

import concourse.bacc as bacc
import concourse.mybir as mybir
import concourse.tile as tile
from concourse.bass_utils import run_bass_kernel_spmd

F32 = mybir.dt.float32
BF16 = mybir.dt.bfloat16
I32 = mybir.dt.int32
AF = mybir.ActivationFunctionType
ALU = mybir.AluOpType

B, S, T = 128, 256, 52
START, STOP, PAD = 50, 51, 0
NCORES = 8
BL = B // NCORES          # 16 sequences per core
GAMMA = 4.0
NORM_TS = (63, 127, 191)  # measure p_t scale at these t
NORM_LAG = 8              # rescale lands on the emission of step t+NORM_LAG
NBANDS = 8                # W computed per band of 32 timesteps
BAND = S // NBANDS        # 32


def build_program(stage=99):
    nc = bacc.Bacc("TRN2", target_bir_lowering=False, debug=False)

    feats_d = nc.dram_tensor("feats128", [128, 1664], F32, kind="ExternalInput")
    tags_d = nc.dram_tensor("tags128", [128, 32], I32, kind="ExternalInput")
    packf_d = nc.dram_tensor("packf", [128, 132], F32, kind="ExternalInput")
    packb_d = nc.dram_tensor("packb", [128, 108], BF16, kind="ExternalInput")
    o_w = nc.dram_tensor("out_w", [S * BL], F32, kind="ExternalOutput")
    o_logs = nc.dram_tensor("out_logs", [3 * BL], F32, kind="ExternalOutput")
    o_gold = nc.dram_tensor("out_gold", [128], F32, kind="ExternalOutput")
    o_dbg = nc.dram_tensor("out_dbg", [128, 16], F32, kind="ExternalOutput")

    with tile.TileContext(nc) as tc:
        with (
            tc.tile_pool(name="sb", bufs=1) as sb,
            tc.tile_pool(name="ptp", bufs=2, space="PSUM") as ptp,
            tc.tile_pool(name="pq", bufs=1, space="PSUM") as pq,
            tc.tile_pool(name="pn", bufs=1, space="PSUM") as pn,
            tc.tile_pool(name="pw", bufs=2, space="PSUM") as pw,
        ):
            # ---- constants (two consolidated packs + tags) ----
            packf = sb.tile([128, 132], F32)
            nc.sync.dma_start(out=packf[:, :], in_=packf_d[:, :])
            packb = sb.tile([128, 108], BF16)
            nc.sync.dma_start(out=packb[:, :], in_=packb_d[:, :])
            tags_sb = sb.tile([128, 32], I32)
            nc.sync.dma_start(out=tags_sb[:, :], in_=tags_d[:, :])

            ident = packf[:, 0:128]
            estart = packf[0:T, 128:129]
            gbias = packf[0:T, 129:130]
            eexp_b = packb[0:T, 0:T]
            estop_b = packb[0:T, T:T + 1]
            onescol_b = packb[0:T, 53:54]
            onesrow_b = packb[0:1, 54:106]

            # ---- feats: 8 column-chunk DMAs so transposes can start early ----
            feats_sb = sb.tile([128, 1664], F32)
            for ci in range(8):
                nc.sync.dma_start(
                    out=feats_sb[:, ci * 208:(ci + 1) * 208],
                    in_=feats_d[:, ci * 208:(ci + 1) * 208],
                )

            if stage <= 1:
                nc.sync.dma_start(out=o_dbg[:, :], in_=feats_sb[:, 0:16])

            # ---- transpose feats -> expf[j, v*128 + b*8 + tc] = exp(f - GAMMA)
            expf = sb.tile([T, 32 * 128], BF16)

            def emit_expf_group(g):
                tp = ptp.tile([T, 512], F32, tag="tp")
                for k in range(4):
                    v = 4 * g + k
                    nc.tensor.transpose(
                        tp[0:T, k * 128:(k + 1) * 128],
                        feats_sb[:, v * T:(v + 1) * T],
                        ident,
                    )
                nc.scalar.activation(
                    expf[0:T, g * 512:(g + 1) * 512], tp[0:T, :], AF.Exp,
                    bias=gbias,
                )

            ngrp_now = 2 if stage >= 4 else (8 if stage >= 2 else 0)
            for g in range(ngrp_now):
                emit_expf_group(g)

            # view: (j, v, b, tc) with t = tc*32 + v
            expf_r = expf[0:T, :].rearrange("p (v b c) -> p v b c", v=32, b=BL, c=8)

            if stage == 2:
                nc.sync.dma_start(out=o_dbg[0:T, :], in_=expf[0:T, 0:16])

            # ---- gold emission gather (independent of scan) ----
            do_gold = (stage >= 3 and stage != 4) or stage in (31, 32)
            jiota = sb.tile([128, 1664], I32)
            onehot = sb.tile([128, 1664], F32)
            goldscr = sb.tile([128, 1664], F32)
            goldp = sb.tile([128, 1], F32)
            if do_gold:
                nc.gpsimd.iota(
                    jiota[:, :].rearrange("p (t j) -> p t j", t=32, j=T),
                    pattern=[[0, 32], [1, T]],
                    base=0,
                    channel_multiplier=0,
                )
                nc.vector.tensor_tensor(
                    out=onehot[:, :].rearrange("p (t j) -> p t j", t=32, j=T),
                    in0=jiota[:, :].rearrange("p (t j) -> p t j", t=32, j=T),
                    in1=tags_sb[:, :].broadcast_to((128, 32, T)),
                    op=ALU.is_equal,
                )
                nc.vector.tensor_tensor(
                    out=goldscr[:, :], in0=onehot[:, :], in1=feats_sb[:, :],
                    op=ALU.mult,
                )
                nc.vector.tensor_reduce(
                    goldp[:, :], goldscr[:, :],
                    axis=mybir.AxisListType.X, op=ALU.add,
                )
                nc.sync.dma_start(out=o_gold[:], in_=goldp[:, :])

            # ---- scan ----
            pall = sb.tile([T, S * BL], BF16)
            logs_sb = sb.tile([1, 3 * BL], F32)
            recip_sb = sb.tile([1, BL], F32)
            recipb_sb = sb.tile([1, BL], BF16)
            wbuf = sb.tile([1, S * BL], F32)

            do_scan = stage >= 4
            do_norm = stage >= 5
            do_w = stage >= 6

            # p_0 = exp(feats_0 - GAMMA) * exp(trans[START, :])
            if do_scan:
                nc.vector.tensor_scalar_mul(
                    pall[0:T, 0:BL], expf_r[0:T, 0, 0:BL, 0], estart
                )

            for t in range(1, S if do_scan else 0):
                v, c = t % 32, t // 32
                if stage >= 4 and t % 4 == 0 and 2 <= t // 4 + 1 <= 7:
                    emit_expf_group(t // 4 + 1)
                for h in range(2):
                    b0 = 8 * h
                    q = pq.tile([T, 8], F32, tag=f"q{h}")
                    nc.tensor.matmul(
                        q[:, :],
                        eexp_b,
                        pall[0:T, (t - 1) * BL + b0:(t - 1) * BL + b0 + 8],
                        start=True, stop=True,
                    )
                    nc.vector.tensor_tensor(
                        out=pall[0:T, t * BL + b0:t * BL + b0 + 8],
                        in0=q[:, :],
                        in1=expf_r[0:T, v, b0:b0 + 8, c],
                        op=ALU.mult,
                    )

                if t in NORM_TS and do_norm:
                    k = NORM_TS.index(t)
                    sn = pn.tile([1, BL], F32, tag="sn")
                    nc.tensor.matmul(
                        sn[0:1, :], onescol_b,
                        pall[0:T, t * BL:(t + 1) * BL],
                        start=True, stop=True,
                    )
                    nc.vector.reciprocal(recip_sb[0:1, :], sn[0:1, :])
                    nc.scalar.copy(recipb_sb[0:1, :], recip_sb[0:1, :])
                    bc = pn.tile([T, BL], F32, tag="bc")
                    nc.tensor.matmul(
                        bc[:, :], onesrow_b, recipb_sb[0:1, :],
                        start=True, stop=True,
                    )
                    # apply the rescale to the emission of step t+NORM_LAG,
                    # off the serial chain (that mult reads it much later)
                    ta = t + NORM_LAG
                    va, ca = ta % 32, ta // 32
                    nc.vector.tensor_tensor(
                        out=expf_r[0:T, va, 0:BL, ca],
                        in0=bc[:, :],
                        in1=expf_r[0:T, va, 0:BL, ca],
                        op=ALU.mult,
                    )
                    # ship log(applied recip scale); host subtracts from c
                    nc.scalar.activation(
                        logs_sb[0:1, k * BL:(k + 1) * BL], recipb_sb[0:1, :],
                        AF.Ln,
                    )

                if t % BAND == BAND - 1 and do_w:
                    n = t // BAND
                    wq = pw.tile([1, BAND * BL], F32, tag="wq")
                    nc.tensor.matmul(
                        wq[0:1, :], estop_b,
                        pall[0:T, n * BAND * BL:(n + 1) * BAND * BL],
                        start=True, stop=True,
                    )
                    nc.scalar.copy(
                        wbuf[0:1, n * BAND * BL:(n + 1) * BAND * BL],
                        wq[0:1, :],
                    )

            if do_scan and not do_w:
                nc.scalar.copy(goldscr[0:T, 0:16], pall[0:T, 100 * BL:101 * BL])
                nc.sync.dma_start(out=o_dbg[0:T, :], in_=goldscr[0:T, 0:16])
            if do_w:
                nc.sync.dma_start(out=o_w[:], in_=wbuf[0:1, :])
                nc.sync.dma_start(out=o_logs[:], in_=logs_sb[0:1, :])

    nc.compile()
    return nc


def host_prep(feats, transitions, mask, tags):
    """Per-core input maps + host-side gold constants."""
    import ml_dtypes
    bf = ml_dtypes.bfloat16

    feats = np.ascontiguousarray(np.asarray(feats, np.float32))
    trans = np.asarray(transitions, np.float32)
    mask = np.asarray(mask).astype(bool)
    tags = np.asarray(tags, np.int32)

    lengths = mask.sum(1).astype(np.int64)                      # (B,)
    eexp = np.exp(trans).astype(np.float32)
    tags_m = np.where(mask, tags, -1).astype(np.int32)

    packf = np.zeros((128, 132), np.float32)
    packf[:, 0:128] = np.eye(128, dtype=np.float32)
    packf[0:T, 128] = np.exp(trans[START]).astype(np.float32)
    packf[0:T, 129] = -GAMMA

    packb = np.zeros((128, 108), np.float32)
    packb[0:T, 0:T] = eexp
    packb[0:T, T] = np.exp(trans[:, STOP]).astype(np.float32)
    packb[0:T, 53] = 1.0
    packb[:, 54:106] = 1.0
    packb = packb.astype(bf)

    in_maps = []
    for core in range(NCORES):
        bs = slice(BL * core, BL * (core + 1))
        f128 = feats[bs].reshape(128, 1664)        # (b,tc,tl,j)->(b*8+tc, tl*52+j)
        t128 = tags_m[bs].reshape(128, 32)
        in_maps.append({
            "feats128": f128,
            "tags128": t128,
            "packf": packf,
            "packb": packb,
        })

    # gold terms that only involve tags/transitions/mask (tiny host math)
    prev = np.concatenate([np.full((B, 1), START, tags.dtype), tags[:, :-1]], 1)
    trans_gold = trans[prev, tags]
    end_ids = tags[np.arange(B), lengths - 1]
    gold_host = np.where(mask, trans_gold, 0.0).sum(dtype=np.float64) \
        + trans[end_ids, STOP].sum(dtype=np.float64)
    return in_maps, lengths, gold_host


def postprocess(results, lengths, gold_host):
    total = np.float64(0.0)
    for core in range(NCORES):
        r = results[core]
        L = lengths[BL * core:BL * (core + 1)].astype(np.float64)
        w = np.asarray(r["out_w"], np.float64).reshape(S, BL)
        selw = w[L.astype(np.int64) - 1, np.arange(BL)]
        logs = np.asarray(r["out_logs"], np.float64).reshape(3, BL)
        gold = np.asarray(r["out_gold"], np.float64)
        cfin = GAMMA * L
        for k, tk in enumerate(NORM_TS):
            # logs holds log(recip) of the applied rescale; c -= log(recip)
            cfin -= (L - 1 >= tk + NORM_LAG) * logs[k]
        term = cfin + np.log(selw)
        total += term.sum() - gold.sum()
    total -= gold_host
    return np.float32(total)


_NC_CACHE = {}


def _get_nc():
    if "nc" not in _NC_CACHE:
        _NC_CACHE["nc"] = build_program()
    return _NC_CACHE["nc"]


def kernel(feats, transitions, mask, tags, _trace=False):
    in_maps, lengths, gold_host = host_prep(feats, transitions, mask, tags)
    nc = _get_nc()
    res = run_bass_kernel_spmd(nc, in_maps, list(range(NCORES)), trace=_trace)
    out = postprocess(res.results, lengths, gold_host)
    if _trace:
        return out, res
    return out
